# revision 1
# baseline (speedup 1.0000x reference)
"""Bilateral filter (nn_BilateralFilter) Trainium2 Bass kernel.

Reference semantics (KERNEL_SIZE=5, THETA_ALPHA=2.0, THETA_BETA=0.1):
    w_k   = exp(-(dx^2+dy^2)/8)                      (24 offsets, center dropped)
    Ki    = exp(-50*(I(p+k) - I(p))^2)               per image channel c
    out[c,n,p] = sum_k w_k*Ki[c,k,p]*Q(n,p+k) / sum_k w_k*Ki[c,k,p]

Sharding: 8 cores = 2 batches x 4 row-slabs of 80 output rows, each slab
shipped with a 2-row halo and 2-col zero padding (84 x 324 per channel).
Device layout: partitions = image rows, free dim = (channel, column).
fp16 on-chip (DVE 2x mode); exponent computed via ACT (Square then in-place
Exp with the spatial weight folded into the per-slot exp bias, plus a +8
exponent shift that keeps fp16 sums well inside normal range and cancels in
the final division).  Compute-engine SBUF accesses spanning >32 partitions
must start at partition 0, so each row shift dr gets its own 80-partition
copy (5 blocks packed in one tile, per-dr DMAs for early start).  Products
are batched over all 5 dc and broadcast over the 6 classes in one
4-dim-AP DVE op per (channel, dr); sums use flat pairwise folds.
Measured: ~199 us on HW (max core), L2 rel err ~6.4e-4 vs the fp32 reference.
"""

import math

import numpy as np

B, C, NCL = 2, 3, 6
H = W = 320
KS, PAD = 5, 2
NK = KS * KS - 1          # 24
WP = W + 2 * PAD          # 324
NSLAB = 4
R = H // NSLAB            # 80 output rows per shard
RH = R + 2 * PAD          # 84 rows incl. halo
COEF = 50.0               # 1/(2*theta_beta^2)
SHIFT = 8.0               # exponent shift, cancels in the division
IW = C * WP               # 972
QW = NCL * WP             # 1944

_CACHE: dict = {}


def _offsets():
    return [
        (dr, dc)
        for dr in range(KS)
        for dc in range(KS)
        if not (dr == PAD and dc == PAD)
    ]


def _emit(tc, i_ap, q_ap, out_ap):
    """Emit the per-core program into TileContext tc.

    i_ap:   DRAM AP (RH, C*WP)  fp16
    q_ap:   DRAM AP (RH, NCL*WP) fp16
    out_ap: DRAM AP (R, C*NCL*W) fp16

    Layout: 25 k-slots (dr-major, center included but killed via a -30
    exp bias so it contributes exactly 0), each slot holding (c, x).
    Products are batched over all 5 dc per (c, dr) in one 4-dim-AP op.
    """
    import concourse.bass as bass
    import concourse.mybir as mybir

    f16 = mybir.dt.float16
    f32 = mybir.dt.float32
    AF = mybir.ActivationFunctionType
    nc = tc.nc
    NS = KS * KS           # 25 slots
    CW = C * W             # 960, slot width in d/sq/kw tiles
    CTR = PAD * KS + PAD   # slot 12 = center

    with tc.tile_pool(name="p", bufs=1) as pool:
        # exp biases as per-partition const columns (activation bias AP)
        bias_vals = {}
        for dr in range(KS):
            for dc in range(KS):
                s = dr * KS + dc
                if s == CTR:
                    bias_vals[s] = SHIFT - 30.0
                else:
                    bias_vals[s] = (
                        SHIFT - ((dr - PAD) ** 2 + (dc - PAD) ** 2) / 8.0
                    )
        distinct = sorted(set(bias_vals.values()))
        bcol = {v: j for j, v in enumerate(distinct)}
        bias_t = pool.tile([R, len(distinct)], f32, tag="bias")
        for v, j in bcol.items():
            nc.vector.memset(bias_t[:, j : j + 1], v)

        # dr processing order: center block first (every sub reads it)
        DRS = [2, 0, 1, 3, 4]

        Ia = pool.tile([R, KS * IW], f16, tag="Ia")
        Qa = pool.tile([R, KS * QW], f16, tag="Qa")
        # per-dr-block DMAs so the first subs/products start early
        for dr in DRS:
            nc.sync.dma_start(
                Ia[:, dr * IW : (dr + 1) * IW], i_ap[dr : dr + R, :]
            )
        for dr in DRS:
            # issue Q loads from the ACT sequencer's DGE (idle at start) so
            # their traffic doesn't delay the I-block completion sems
            nc.scalar.dma_start(
                Qa[:, dr * QW : (dr + 1) * QW], q_ap[dr : dr + R, :]
            )

        def i_v(dr, dc):
            # [R, (c,320)] view of I at offset (dr, dc); c-stride WP
            return bass.AP(
                tensor=Ia.tensor, offset=Ia.offset + dr * IW + dc,
                ap=[[KS * IW, R], [WP, C], [1, W]],
            )

        # d[slot] = I(p+k) - I(p), all 3 channels per op; ACT square + exp
        # interleaved per dr so the first products unblock early
        d = pool.tile([R, NS * CW], f16, tag="big0")
        sq = pool.tile([R, NS * CW], f16, tag="big1")
        kw = sq  # exp runs in place over sq (elementwise ACT stream)
        for dr in DRS:
            # one 4-dim-AP sub for all 5 dc x 3 c; center slot yields 0
            lo, hi = dr * KS * CW, (dr + 1) * KS * CW
            dst = d[:, lo:hi].rearrange("p (dc c w) -> p dc c w", dc=KS, c=C)
            in0 = bass.AP(
                tensor=Ia.tensor, offset=Ia.offset + dr * IW,
                ap=[[KS * IW, R], [1, KS], [WP, C], [1, W]],
            )
            in1 = bass.AP(
                tensor=Ia.tensor, offset=Ia.offset + PAD * IW + PAD,
                ap=[[KS * IW, R], [0, KS], [WP, C], [1, W]],
            )
            nc.vector.tensor_sub(dst, in0, in1)
            nc.scalar.activation(sq[:, lo:hi], d[:, lo:hi], AF.Square)
            for dc in range(KS):
                s = dr * KS + dc
                j = bcol[bias_vals[s]]
                nc.scalar.activation(
                    kw[:, s * CW : (s + 1) * CW],
                    sq[:, s * CW : (s + 1) * CW],
                    AF.Exp,
                    bias=bias_t[:, j : j + 1],
                    scale=-COEF,
                )

        # Products dr-outer / c-inner: each exp group (one dr) feeds ~3x
        # more DVE work, so DVE never stalls on the ACT exp pipeline.
        NW = NCL * W
        accs = [pool.tile([R, NCL * W], f16, tag=f"acc{c}", name=f"acc{c}") for c in range(C)]
        for dr in DRS:
            for c in range(C):
                acc = accs[c]
                # P5[dc, n, x] = kw[5dr+dc, c, x] * Q[n, p+(dr,dc-2)]
                P5 = pool.tile([R, KS * NCL * W], f16, tag="P5", bufs=1)
                if dr == PAD:
                    # center slot is zero - compute only taps {10,11},{13,14}
                    for h, (s0, qo) in enumerate(((10, 0), (13, 3))):
                        kw_src = bass.AP(
                            tensor=kw.tensor,
                            offset=kw.offset + s0 * CW + c * W,
                            ap=[[NS * CW, R], [CW, 2], [0, NCL], [1, W]],
                        )
                        q_src = bass.AP(
                            tensor=Qa.tensor,
                            offset=Qa.offset + dr * QW + qo,
                            ap=[[KS * QW, R], [1, 2], [WP, NCL], [1, W]],
                        )
                        nc.vector.tensor_mul(
                            P5[:, h * 2 * NW : (h + 1) * 2 * NW].rearrange(
                                "p (dc n w) -> p dc n w", dc=2, n=NCL
                            ),
                            kw_src,
                            q_src,
                        )
                    s1 = pool.tile([R, 2 * NW], f16, tag="s1", bufs=1)
                    nc.vector.tensor_add(
                        s1[:, :], P5[:, : 2 * NW], P5[:, 2 * NW : 4 * NW]
                    )
                    # dr=PAD is first in DRS: fold straight into acc
                    nc.vector.tensor_add(acc[:, :], s1[:, :NW], s1[:, NW:])
                    continue
                kw_src = bass.AP(
                    tensor=kw.tensor,
                    offset=kw.offset + (dr * KS) * CW + c * W,
                    ap=[[NS * CW, R], [CW, KS], [0, NCL], [1, W]],
                )
                q_src = bass.AP(
                    tensor=Qa.tensor, offset=Qa.offset + dr * QW,
                    ap=[[KS * QW, R], [1, KS], [WP, NCL], [1, W]],
                )
                nc.vector.tensor_mul(
                    P5[:, :].rearrange("p (dc n w) -> p dc n w", dc=KS, n=NCL),
                    kw_src,
                    q_src,
                )
                # fold 5 -> 1: [A+C, B+D] ; + ; + E
                s1 = pool.tile([R, 2 * NW], f16, tag="s1", bufs=1)
                nc.vector.tensor_add(
                    s1[:, :], P5[:, : 2 * NW], P5[:, 2 * NW : 4 * NW]
                )
                s2 = pool.tile([R, NW], f16, tag="s2", bufs=1)
                nc.vector.tensor_add(s2[:, :], s1[:, :NW], s1[:, NW:])
                s3 = pool.tile([R, NW], f16, tag="s1", bufs=1)
                nc.vector.tensor_add(s3[:, :], s2[:, :], P5[:, 4 * NW :])
                nc.vector.tensor_add(acc[:, :], acc[:, :], s3[:, :])

        # norm for all channels at once, in kw's native [slot,(c,x)] layout:
        # flat pairwise folds 24 -> 12 -> 6 -> 3 -> 1 (+ zero center slot).
        nt_a = pool.tile([R, 12 * CW], f16, tag="big0")
        nc.vector.tensor_add(
            nt_a[:, :], kw[:, : 12 * CW], kw[:, 12 * CW : 24 * CW]
        )
        nt_b = pool.tile([R, 6 * CW], f16, tag="nt_b")
        nc.vector.tensor_add(nt_b[:, :], nt_a[:, : 6 * CW], nt_a[:, 6 * CW :])
        nt_c = pool.tile([R, 3 * CW], f16, tag="nt_c")
        nc.vector.tensor_add(nt_c[:, :], nt_b[:, : 3 * CW], nt_b[:, 3 * CW :])
        n1 = pool.tile([R, CW], f16, tag="n1")
        nc.vector.tensor_add(n1[:, :], nt_c[:, :CW], nt_c[:, CW : 2 * CW])
        nc.vector.tensor_add(n1[:, :], n1[:, :], nt_c[:, 2 * CW : 3 * CW])
        norm = pool.tile([R, CW], f32, tag="norm")
        nc.vector.tensor_add(norm[:, :], n1[:, :], kw[:, 24 * CW : 25 * CW])
        rnorm = pool.tile([R, CW], f32, tag="rnorm")
        nc.vector.reciprocal_approx_fast(rnorm[:, :], norm[:, :])
        rnh = pool.tile([R, CW], f16, tag="rnh")
        nc.vector.tensor_copy(rnh[:, :], rnorm[:, :])

        for c in range(C):
            acc = accs[c]
            ot = pool.tile([R, NCL * W], f16, tag="out", bufs=2)
            rb = (
                rnh[:, c * W : (c + 1) * W]
                .unsqueeze(1)
                .broadcast_to([R, NCL, W])
            )
            nc.vector.tensor_mul(
                ot[:, :].rearrange("p (n w) -> p n w", n=NCL),
                acc[:, :].rearrange("p (n w) -> p n w", n=NCL),
                rb,
            )
            nc.sync.dma_start(
                out_ap[:, c * NCL * W : (c + 1) * NCL * W], ot[:, :]
            )


def _build_program():
    import concourse.bacc as bacc
    import concourse.mybir as mybir
    from concourse import tile

    f16 = mybir.dt.float16

    nc = bacc.Bacc("TRN2", num_devices=8, debug=False)
    I_in = nc.dram_tensor("i_in", [RH, IW], f16, kind="ExternalInput")
    Q_in = nc.dram_tensor("q_in", [RH, QW], f16, kind="ExternalInput")
    OUT = nc.dram_tensor("out", [R, C * NCL * W], f16, kind="ExternalOutput")

    with tile.TileContext(nc) as tc:
        _emit(tc, I_in.ap(), Q_in.ap(), OUT.ap())

    nc.compile()
    return nc


def _get_program():
    if "nc" not in _CACHE:
        _CACHE["nc"] = _build_program()
    return _CACHE["nc"]


def _shard_inputs(Q, I):
    """Host prep: pad, cast fp16, per-shard (rows, chan*cols) layout."""
    Qp = np.pad(
        np.asarray(Q, np.float32), ((0, 0), (0, 0), (PAD, PAD), (PAD, PAD))
    ).astype(np.float16)
    Ip = np.pad(
        np.asarray(I, np.float32), ((0, 0), (0, 0), (PAD, PAD), (PAD, PAD))
    ).astype(np.float16)
    in_maps = []
    for b in range(B):
        for s in range(NSLAB):
            r0 = s * R
            i_sh = Ip[b, :, r0 : r0 + RH, :]  # (C, RH, WP)
            q_sh = Qp[b, :, r0 : r0 + RH, :]  # (NCL, RH, WP)
            in_maps.append(
                {
                    "i_in": np.ascontiguousarray(
                        i_sh.transpose(1, 0, 2).reshape(RH, IW)
                    ),
                    "q_in": np.ascontiguousarray(
                        q_sh.transpose(1, 0, 2).reshape(RH, QW)
                    ),
                }
            )
    return in_maps


def _assemble(outs):
    # outs: list of 8 arrays (R, C*NCL*W), core order = (b, slab)
    o = np.stack([np.asarray(x) for x in outs]).astype(np.float32)
    o = o.reshape(B, NSLAB, R, C, NCL, W)
    o = o.transpose(0, 3, 4, 1, 2, 5).reshape(B, C, NCL, H, W)
    return o


def run(Q, I, trace=False):
    from concourse.bass_utils import run_bass_kernel_spmd

    nc = _get_program()
    in_maps = _shard_inputs(Q, I)
    res = run_bass_kernel_spmd(nc, in_maps, list(range(8)), trace=trace)
    out = _assemble([res.results[i]["out"] for i in range(8)])
    return out, res


def kernel(Q, I):
    out, _ = run(Q, I)
    return out



# revision 5
# speedup vs baseline: 1.8013x; 1.8013x over previous
"""Bilateral filter (nn_BilateralFilter) Trainium2 Bass kernel.

Reference semantics (KERNEL_SIZE=5, THETA_ALPHA=2.0, THETA_BETA=0.1):
    w_k   = exp(-(dx^2+dy^2)/8)                      (24 offsets, center dropped)
    Ki    = exp(-50*(I(p+k) - I(p))^2)               per image channel c
    out[c,n,p] = sum_k w_k*Ki[c,k,p]*Q(n,p+k) / sum_k w_k*Ki[c,k,p]

Sharding: 8 cores = 2 batches x 4 row-slabs of 80 output rows.  On-chip the
slab is processed as 2 half-slabs with partitions = (3 channels x 40 rows)
= 120 of 128 lanes (Q is replicated 3x across the channel blocks by DMA).

Engine split (vs the all-DVE baseline):
  DVE : neighbor subs + the 24x(c,n)-product planes (fp16, 2x mode)
  ACT : Square + Exp with the spatial weight folded into per-group exp bias
        (slots grouped by (dr^2+dc^2) so one strided activation covers a
        +/-dc pair); +8 exponent shift cancels in the final division
  PE  : all k-fold reductions as identity-weight matmuls accumulating into
        PSUM fp32 (6 numerator banks = one per class, 1 norm bank per half)
  DMA : input loads on the SP/GPSIMD queues, which are otherwise idle

The center slot is excluded from both folds (reference drops it); its kw
value stays exactly 0 because d=0 -> Square -> 0 and Exp is never applied.
"""

import numpy as np

B, C, NCL = 2, 3, 6
H = W = 320
KS, PAD = 5, 2
WP = W + 2 * PAD          # 324
NSLAB = 4
R = H // NSLAB            # 80 output rows per shard
RH = R + 2 * PAD          # 84 rows incl. halo
HALF = R // 2             # 40 rows per half-slab
NP = C * HALF             # 120 partitions
COEF = 50.0               # 1/(2*theta_beta^2)
SHIFT = 8.0               # exponent shift, cancels in the division
IWH = KS * WP             # Ia free width per half: 5 dr blocks  (1620)
QWH = KS * NCL * WP       # Qa free width per half: 5 dr x 6 n   (9720)
DW = KS * W               # d/sq free width per dr block = 5*320 (1600)
SW = KS * KS * W          # sq tile free width, 25 slots         (8000)
PW = KS * NCL * W         # P5 free width: 5 dc x 6 n x 320      (9600)
NW = NCL * W              # 1920

_CACHE: dict = {}

# dr processing order: center block first (every sub reads it)
DRS = [2, 0, 1, 3, 4]


def _emit(tc, i_ap, q_ap, e_ap, out_ap):
    """Emit the per-core program into TileContext tc.

    i_ap:   DRAM AP (C*RH*WP,)  fp16   image, zero-padded
    q_ap:   DRAM AP (NCL*RH*WP,) fp16  unaries, zero-padded
    e_ap:   DRAM AP (NP*NP,) fp16      identity matrix
    out_ap: DRAM AP (R*C*NCL*W,) fp16  output rows x (c, n, x)
    """
    import concourse.bass as bass
    import concourse.mybir as mybir

    f16 = mybir.dt.float16
    f32 = mybir.dt.float32
    AF = mybir.ActivationFunctionType
    nc = tc.nc

    # slots excluded from folds: the center (dr=2, dc=2)
    def fold_slots():
        return [
            (dr, dc)
            for dr in DRS
            for dc in range(KS)
            if not (dr == PAD and dc == PAD)
        ]

    with tc.tile_pool(name="p", bufs=1) as pool, tc.tile_pool(
        name="pp", bufs=1, space="PSUM"
    ) as ppool:
        # exp biases: b = SHIFT - s/8 for s = (dr-2)^2 + (dc-2)^2
        svals = [1, 2, 4, 5, 8]
        bias_t = pool.tile([NP, len(svals)], f32, tag="bias")
        bcol = {}
        for j, s in enumerate(svals):
            bcol[s] = j
            nc.gpsimd.memset(bias_t[:, j : j + 1], SHIFT - s / 8.0)

        ident = pool.tile([NP, NP], f16, tag="ident")
        nc.gpsimd.dma_start(ident[:, :], e_ap)

        # input loads for both halves up front; the host ships i_in/q_in
        # already in on-chip layout [h, dr, (c,rr), free] so every load is a
        # contiguous 2-dim copy.  I on the SP queue, Q on the GPSIMD queue
        # (both sequencers otherwise idle).
        Ia = [pool.tile([NP, IWH], f16, tag=f"Ia{h}", name=f"Ia{h}") for h in range(2)]
        Qa = [pool.tile([NP, QWH], f16, tag=f"Qa{h}", name=f"Qa{h}") for h in range(2)]
        for h in range(2):
            for dr in DRS:
                nc.sync.dma_start(
                    Ia[h][:, dr * WP : (dr + 1) * WP],
                    bass.AP(
                        tensor=i_ap.tensor,
                        offset=i_ap.offset + (h * KS + dr) * NP * WP,
                        ap=[[WP, NP], [1, WP]],
                    ),
                )
        for h in range(2):
            for dr in DRS:
                nc.gpsimd.dma_start(
                    Qa[h][:, dr * NCL * WP : (dr + 1) * NCL * WP],
                    bass.AP(
                        tensor=q_ap.tensor,
                        offset=q_ap.offset + (h * KS + dr) * NP * NCL * WP,
                        ap=[[NCL * WP, NP], [1, NCL * WP]],
                    ),
                )

        sq = [pool.tile([NP, SW], f16, tag=f"sq{h}", name=f"sq{h}") for h in range(2)]
        d_t = pool.tile([NP, SW], f16, tag="d")  # shared scratch across halves

        def emit_front(h):
            """subs (DVE) + square/exp (ACT) for half h; kw ends up in sq[h]."""
            ia, sqh = Ia[h], sq[h]
            for dr in DRS:
                # d[dr block] = I(p + (dr, dc)) - I(p), all 5 dc in one op
                dst = bass.AP(
                    tensor=d_t.tensor,
                    offset=d_t.offset + dr * DW,
                    ap=[[SW, NP], [W, KS], [1, W]],
                )
                in0 = bass.AP(
                    tensor=ia.tensor,
                    offset=ia.offset + dr * WP,
                    ap=[[IWH, NP], [1, KS], [1, W]],
                )
                in1 = bass.AP(
                    tensor=ia.tensor,
                    offset=ia.offset + PAD * WP + PAD,
                    ap=[[IWH, NP], [0, KS], [1, W]],
                )
                nc.vector.tensor_sub(dst, in0, in1)
            for dr in DRS:
                nc.scalar.activation(
                    sqh[:, dr * DW : (dr + 1) * DW],
                    d_t[:, dr * DW : (dr + 1) * DW],
                    AF.Square,
                )
                # exp in place, slots grouped by |dc-2|: {0,4}, {1,3}, {2}
                for dcs, ds2 in (((0, 4), 4), ((1, 3), 1), (((2,)), 0)):
                    if dr == PAD and ds2 == 0:
                        continue  # center slot stays 0
                    s = (dr - PAD) ** 2 + ds2
                    j = bcol[s]
                    if len(dcs) == 2:
                        ap_dims = [[SW, NP], [(dcs[1] - dcs[0]) * W, 2], [1, W]]
                    else:
                        ap_dims = [[SW, NP], [1, W]]
                    src = bass.AP(
                        tensor=sqh.tensor,
                        offset=sqh.offset + dr * DW + dcs[0] * W,
                        ap=ap_dims,
                    )
                    nc.scalar.activation(
                        src, src, AF.Exp, bias=bias_t[:, j : j + 1], scale=-COEF
                    )

        def emit_products(h, psum_n, psum_nrm):
            """products (DVE) + fold matmuls (PE) for half h."""
            kw, qa = sq[h], Qa[h]
            slots = fold_slots()
            first, last = slots[0], slots[-1]
            for dr in DRS:
                p5 = pool.tile([NP, PW], f16, tag="P5", bufs=2)
                out = bass.AP(
                    tensor=p5.tensor,
                    offset=p5.offset,
                    ap=[[PW, NP], [NW, KS], [W, NCL], [1, W]],
                )
                in0 = bass.AP(
                    tensor=kw.tensor,
                    offset=kw.offset + dr * DW,
                    ap=[[SW, NP], [W, KS], [0, NCL], [1, W]],
                )
                in1 = bass.AP(
                    tensor=qa.tensor,
                    offset=qa.offset + dr * NCL * WP,
                    ap=[[QWH, NP], [1, KS], [WP, NCL], [1, W]],
                )
                nc.vector.tensor_mul(out, in0, in1)
                # fold this dr's slots into PSUM on the PE
                for dc in range(KS):
                    if dr == PAD and dc == PAD:
                        continue
                    st = (dr, dc) == first
                    sp = (dr, dc) == last
                    for n in range(NCL):
                        nc.tensor.matmul(
                            psum_n[n][:, :],
                            ident[:, :],
                            p5[:, dc * NW + n * W : dc * NW + (n + 1) * W],
                            start=st,
                            stop=sp,
                        )
                    nc.tensor.matmul(
                        psum_nrm[:, :],
                        ident[:, :],
                        kw[:, (dr * KS + dc) * W : (dr * KS + dc + 1) * W],
                        start=st,
                        stop=sp,
                    )

        def emit_readout(h, psum_n, psum_nrm):
            """recip + final scale (DVE) and the output store."""
            rnorm = pool.tile([NP, W], f32, tag=f"rn{h}")
            nc.vector.reciprocal_approx_fast(rnorm[:, :], psum_nrm[:, :])
            ot = pool.tile([NP, NW], f16, tag=f"ot{h}")
            for n in range(NCL):
                nc.vector.tensor_mul(
                    ot[:, n * W : (n + 1) * W], psum_n[n][:, :], rnorm[:, :]
                )
            nc.gpsimd.dma_start(
                bass.AP(
                    tensor=out_ap.tensor,
                    offset=out_ap.offset + h * HALF * C * NW,
                    ap=[[NW, C], [C * NW, HALF], [1, NW]],
                ),
                ot[:, :],
            )

        # PSUM tiles shared across halves (bufs=1): half B's first matmul on
        # a bank waits for half A's readout of that bank -> per-bank pipelining
        psum_n = [ppool.tile([NP, W], f32, tag=f"ps{n}", name=f"ps{n}") for n in range(NCL)]
        psum_nrm = ppool.tile([NP, W], f32, tag="psn")

        emit_front(0)
        emit_products(0, psum_n, psum_nrm)
        emit_front(1)
        emit_readout(0, psum_n, psum_nrm)
        emit_products(1, psum_n, psum_nrm)
        emit_readout(1, psum_n, psum_nrm)


def _build_program():
    import concourse.bacc as bacc
    import concourse.mybir as mybir
    from concourse import tile

    f16 = mybir.dt.float16

    nc = bacc.Bacc("TRN2", num_devices=8, debug=False)
    I_in = nc.dram_tensor("i_in", [2 * KS * NP * WP], f16, kind="ExternalInput")
    Q_in = nc.dram_tensor("q_in", [2 * KS * NP * NCL * WP], f16, kind="ExternalInput")
    E_in = nc.dram_tensor("ident", [NP, NP], f16, kind="ExternalInput")
    OUT = nc.dram_tensor("out", [R * C * NCL * W], f16, kind="ExternalOutput")

    with tile.TileContext(nc) as tc:
        _emit(tc, I_in.ap(), Q_in.ap(), E_in.ap(), OUT.ap())

    nc.compile()
    return nc


def _get_program():
    if "nc" not in _CACHE:
        _CACHE["nc"] = _build_program()
    return _CACHE["nc"]


def _shard_inputs(Q, I):
    """Host prep: pad, cast fp16, and pre-lay each core's inputs in the exact
    on-chip layout [half, dr, (c,rr) partitions, free] so device DMAs are
    contiguous.  Q rows are replicated across the 3 channel blocks."""
    Qp = np.pad(
        np.asarray(Q, np.float32), ((0, 0), (0, 0), (PAD, PAD), (PAD, PAD))
    ).astype(np.float16)
    Ip = np.pad(
        np.asarray(I, np.float32), ((0, 0), (0, 0), (PAD, PAD), (PAD, PAD))
    ).astype(np.float16)
    eye = np.eye(NP, dtype=np.float16)
    in_maps = []
    for b in range(B):
        for s in range(NSLAB):
            r0 = s * R
            i_blk = np.empty((2, KS, NP, WP), np.float16)
            q_blk = np.empty((2, KS, NP, NCL * WP), np.float16)
            for h in range(2):
                for dr in range(KS):
                    rr = r0 + h * HALF + dr
                    # I: partitions = (c, row) c-major
                    i_blk[h, dr] = Ip[b, :, rr : rr + HALF, :].reshape(NP, WP)
                    # Q: (n, rr, x) -> (rr, n, x), replicated over c
                    qb = Qp[b, :, rr : rr + HALF, :].transpose(1, 0, 2)
                    q_blk[h, dr] = np.tile(
                        qb.reshape(1, HALF, NCL * WP), (C, 1, 1)
                    ).reshape(NP, NCL * WP)
            in_maps.append(
                {
                    "i_in": i_blk.reshape(-1),
                    "q_in": q_blk.reshape(-1),
                    "ident": eye,
                }
            )
    return in_maps


def _assemble(outs):
    # outs: list of 8 arrays (R*C*NCL*W,), core order = (b, slab)
    o = np.stack([np.asarray(x) for x in outs]).astype(np.float32)
    o = o.reshape(B, NSLAB, R, C, NCL, W)
    o = o.transpose(0, 3, 4, 1, 2, 5).reshape(B, C, NCL, H, W)
    return o


def run(Q, I, trace=False):
    from concourse.bass_utils import run_bass_kernel_spmd

    nc = _get_program()
    in_maps = _shard_inputs(Q, I)
    res = run_bass_kernel_spmd(nc, in_maps, list(range(8)), trace=trace)
    out = _assemble([res.results[i]["out"] for i in range(8)])
    return out, res


def kernel(Q, I):
    out, _ = run(Q, I)
    return out
